# revision 42
# baseline (speedup 1.0000x reference)
"""Trainium2 Bass kernel for AdaptiveNet MLP (fc1+sigmoid, grouped fc2+sigmoid, fc3).

Sharding: pure data-parallel over batch across 8 NeuronCores (no collectives).
Each core computes its 2048-row shard through all three layers.

fc1 (95% of FLOPs) runs in fp8-e4m3 with DoubleRow perf mode (two fp8 weights
per PE cell -> K=256 per matmul, halving the matmul count); the sigmoid damps
the quantization error so the final rel-err stays ~3e-3 (gate is 2e-2).

Layout trick: H1 is permuted s-major on the host (h1' = s*512 + g, where the
original h1 = g*8 + s).  fc1 then produces hT' tiles [128 h1' partitions x 512
rows]; the grouped fc2 contraction over s becomes 8 fused multiply-accumulate
ops on the vector engine with per-partition scalars (W2 columns), and fc3 is a
plain bf16 matmul over the 512 groups.  Biases are per-partition [128,1]
columns fused into ScalarE sigmoids / a VectorE add.
"""

import sys

for _p in ("/opt/trn_rl_repo",):
    if _p not in sys.path:
        sys.path.append(_p)

import numpy as np
import ml_dtypes

BF16 = ml_dtypes.bfloat16
FP8 = ml_dtypes.float8_e4m3  # == mybir.dt.float8e4

D_IN, H1, H2, D_OUT = 1024, 4096, 512, 256
GS = H1 // H2  # 8
B = 16384
N_CORES = 8
B_SHARD = B // N_CORES  # 2048
NBLK = 512  # rows per block (one PSUM bank of fp32)
NB = B_SHARD // NBLK  # 4
KC = D_IN // 128  # 8 contraction subtiles for fc1
KP = KC // 2  # 4 DoubleRow pairs
CC = H1 // 128  # 32 h1' chunks
NT = H2 // 128  # 4 x2T tiles
ND = D_OUT // 128  # 2 output chunks
W3SCALE = 16.0  # fp8 W3 pre-scale (avoids subnormals); undone in the bias add

_compiled = {}


def _build_nc():
    from concourse import bacc, tile, mybir

    f32 = mybir.dt.float32
    bf16 = mybir.dt.bfloat16
    fp8 = mybir.dt.float8e4
    AF = mybir.ActivationFunctionType
    ALU = mybir.AluOpType
    DR = mybir.MatmulPerfMode.DoubleRow

    nc = bacc.Bacc("TRN2", target_bir_lowering=False, debug=False,
                   num_devices=N_CORES)

    xq = nc.dram_tensor("xq", [128, KC, B_SHARD], fp8, kind="ExternalInput")
    w1q = nc.dram_tensor("w1q", [128, KC, H1], fp8, kind="ExternalInput")
    # all [128, *] f32 constants packed on the free axis:
    # b1 (CC) | w2 (CC) | b2 (NT) | b3 (ND)
    cst = nc.dram_tensor("cst", [128, 2 * CC + NT + ND], f32,
                         kind="ExternalInput")
    w3q = nc.dram_tensor("w3q", [128, NT, D_OUT], fp8, kind="ExternalInput")
    out = nc.dram_tensor("out", [D_OUT, B_SHARD], bf16, kind="ExternalOutput")

    with tile.TileContext(nc) as tc:
        with (
            tc.tile_pool(name="wpool", bufs=1) as wpool,
            tc.tile_pool(name="xpool", bufs=1) as xpool,
            tc.tile_pool(name="hpool", bufs=8) as hpool,
            tc.tile_pool(name="accpool", bufs=1) as accpool,
            tc.tile_pool(name="x2pool", bufs=1) as x2pool,
            tc.tile_pool(name="opool", bufs=4) as opool,
            tc.tile_pool(name="psum_h", bufs=8, space="PSUM") as psum_h_pool,
        ):
            psum_o_pool = psum_h_pool
            w1_sb = wpool.tile([128, KC, H1], fp8, tag="w1")
            x_sb = [None] * NB
            for n in range(NB):
                x_sb[n] = xpool.tile([128, KC, NBLK], fp8,
                                     tag=f"x_{n}", name=f"xsb_{n}")
            # Three issuing queues (sync/scalar HWDGE, gpsimd SWDGE), FIFO
            # within each; ~2-3us per dma_start regardless of size, so use
            # few DMAs, landing in exactly the order the loop consumes:
            # W1 column-blocks (all subtile pairs of CBLK c-chunks in one
            # strided DMA) on sync; whole x tiles on scalar/gpsimd.
            CBLK = 4
            WBLK = 8

            cst_sb = wpool.tile([128, 2 * CC + NT + ND], f32, tag="cst")
            w3_sb = wpool.tile([128, NT, D_OUT], fp8, tag="w3q")

            def b1col(c):
                return cst_sb[:, c:c + 1]

            def w2col(c):
                return cst_sb[:, CC + c:CC + c + 1]

            def b2col(t):
                return cst_sb[:, 2 * CC + t:2 * CC + t + 1]

            def b3col(d):
                return cst_sb[:, 2 * CC + NT + d:2 * CC + NT + d + 1]

            # one DMA per W1 column-block covering all subtile pairs; the
            # first block split in half so the very first matmuls unblock
            # sooner
            def wblock(cb0, cb1, eng):
                c0, c1 = cb0 * 128, cb1 * 128
                eng.dma_start(w1_sb[:, :, c0:c1], w1q.ap()[:, :, c0:c1])

            def xdma(n, eng):
                eng.dma_start(x_sb[n][:],
                              xq.ap()[:, :, n * NBLK:(n + 1) * NBLK])

            # sync lane: W1 first half-block x2, then x_1/x_3 slotted in
            # before the later blocks (which have plenty of slack)
            wblock(0, CBLK // 2, nc.sync)
            wblock(CBLK // 2, CBLK, nc.sync)
            xdma(1, nc.sync)
            wblock(CBLK, 2 * CBLK, nc.sync)
            xdma(3, nc.sync)
            for cb in range(2 * CBLK, CC, WBLK):
                wblock(cb, cb + WBLK, nc.sync)
            # scalar/gpsimd lanes: x_0 split across both lanes so the
            # first chunk's matmuls unblock ~1.5us sooner, then x_2/consts
            nc.scalar.dma_start(x_sb[0][:, 0:KC // 2, :],
                                xq.ap()[:, 0:KC // 2, 0:NBLK])
            nc.gpsimd.dma_start(x_sb[0][:, KC // 2:KC, :],
                                xq.ap()[:, KC // 2:KC, 0:NBLK])
            nc.scalar.dma_start(cst_sb[:], cst.ap()[:])
            xdma(2, nc.gpsimd)
            nc.gpsimd.dma_start(w3_sb[:], w3q.ap()[:])

            # fc2 accumulators, one per (row-block, x2 tile)
            acc = [[None] * NT for _ in range(NB)]

            # x2 packed per row-block: [128, NT, NBLK] fp8, centered by
            # -0.5 so e4m3 quantization error stays small; the 0.5*sum(W3)
            # correction is folded into b3 on the host
            x2_sb = [None] * NB
            for n in range(NB):
                x2_sb[n] = x2pool.tile([128, NT, NBLK], fp8,
                                       tag=f"x2_{n}", name=f"x2p_{n}")

            def fc2_step(c, n, ht):
                t_i = c % NT
                if c < NT:
                    acc[n][t_i] = accpool.tile([128, NBLK], bf16,
                                               tag=f"acc_{n}_{t_i}",
                                               name=f"acc_{n}_{t_i}")
                    nc.vector.tensor_scalar_mul(acc[n][t_i][:], ht[:],
                                                w2col(c))
                else:
                    nc.vector.scalar_tensor_tensor(
                        acc[n][t_i][:], ht[:], w2col(c),
                        acc[n][t_i][:], op0=ALU.mult, op1=ALU.add)
                if c >= CC - NT:
                    # chain for tile t_i is complete -> fc2 sigmoid now,
                    # then center by -0.5 into the packed fp8 x2 tile
                    x2h = hpool.tile([128, NBLK], bf16, tag="x2h",
                                     name=f"x2h_{n}_{t_i}")
                    nc.scalar.activation(x2h[:], acc[n][t_i][:], AF.Sigmoid,
                                         bias=b2col(t_i))
                    nc.vector.tensor_scalar_sub(
                        x2_sb[n][:, t_i, :], x2h[:], 0.5)

            def fc1_block(cs, n):
                for c in cs:
                    ph = psum_h_pool.tile([128, NBLK], f32, tag="psum_h",
                                          name=f"ph_{n}_{c}")
                    for j in range(KP):
                        nc.tensor.matmul(
                            ph[:],
                            lhsT=w1_sb[:, 2 * j:2 * j + 2,
                                       128 * c:128 * (c + 1)],
                            rhs=x_sb[n][:, 2 * j:2 * j + 2, :],
                            start=(j == 0),
                            stop=(j == KP - 1),
                            perf_mode=DR,
                        )
                    ht = hpool.tile([128, NBLK], bf16, tag="ht",
                                    name=f"ht_{n}_{c}")
                    nc.scalar.activation(ht[:], ph[:], AF.Sigmoid,
                                         bias=b1col(c))
                    fc2_step(c, n, ht)

            po_t = {}

            def fc3_jj(n, jj):
                for d in range(ND):
                    if jj == 0:
                        po_t[(n, d)] = psum_o_pool.tile(
                            [128, NBLK], f32, tag="psum_h",
                            name=f"po_{n}_{d}")
                    nc.tensor.matmul(
                        po_t[(n, d)][:],
                        lhsT=w3_sb[:, 2 * jj:2 * jj + 2,
                                   128 * d:128 * (d + 1)],
                        rhs=x2_sb[n][:, 2 * jj:2 * jj + 2, :],
                        start=(jj == 0),
                        stop=(jj == NT // 2 - 1),
                        perf_mode=DR,
                    )

            def fc3_out(n):
                for d in range(ND):
                    ot = opool.tile([128, NBLK], bf16, tag="ot",
                                    name=f"ot_{n}_{d}")
                    nc.vector.tensor_scalar(ot[:], po_t[(n, d)][:],
                                            1.0 / W3SCALE, b3col(d),
                                            op0=ALU.mult, op1=ALU.add)
                    nc.sync.dma_start(
                        out.ap()[128 * d:128 * (d + 1),
                                 n * NBLK:(n + 1) * NBLK], ot[:])

            # --- fc1 + fc2: column-blocks of CBLK c-chunks, n-outer inside
            # so each x tile's DMA arrival unlocks a block of work; each
            # psum tile's 4 matmuls are consecutive (liveness ~1 bank).
            # In the last block, each row-block's fc3 is interleaved one
            # n-phase behind its fc1 so the x2 sigmoid chains are covered
            # by other matmul work. ---
            NORD = (0, 2, 1, 3)  # x DMA arrival order (2 lanes)
            # chain updates are commutative, so interleave the final c's
            # over the last two blocks: x2 sigmoids spread across 8 chunks
            # instead of bunching behind the last 4 (ACT would throttle PE
            # via PSUM slot release)
            c_seq = list(range(CC - 2 * CBLK)) + [24, 28, 25, 29, 26, 30,
                                                 27, 31]
            for b0 in range(0, CC, CBLK):
                for n in NORD:
                    fc1_block(c_seq[b0:b0 + CBLK], n)
            # jj=0 (x2 tiles 0,1 — ready early) for all row blocks first,
            # covering the tail of the t=2,3 sigmoid chains
            for n in NORD:
                fc3_jj(n, 0)
            for n in NORD:
                fc3_jj(n, 1)
                fc3_out(n)

    nc.compile()
    return nc


def get_nc():
    if "nc" not in _compiled:
        _compiled["nc"] = _build_nc()
    return _compiled["nc"]


def make_in_maps(x, W1, b1, W2, b2, W3, b3):
    x = np.asarray(x, dtype=np.float32)
    W1 = np.asarray(W1, dtype=np.float32)
    b1 = np.asarray(b1, dtype=np.float32)
    W2 = np.asarray(W2, dtype=np.float32)
    b2 = np.asarray(b2, dtype=np.float32)
    W3 = np.asarray(W3, dtype=np.float32)
    b3 = np.asarray(b3, dtype=np.float32)

    # s-major permutation of H1: new index p = s*H2 + g  (old h1 = g*GS + s)
    p = np.arange(H1)
    perm = (p % H2) * GS + (p // H2)
    W1p = W1[perm, :]
    b1p = b1[perm]

    # fp8 fc1 operands in DoubleRow layout [128, KC, *]:
    # element (p, j, m) holds contraction index k = 128*j + p
    w1t = W1p.T.astype(FP8)  # [D_IN, H1]
    w1q_h = np.ascontiguousarray(
        w1t.reshape(KC, 128, H1).transpose(1, 0, 2))
    xt = x.T.astype(FP8)  # [D_IN, B]
    xq_h = np.ascontiguousarray(
        xt.reshape(KC, 128, B).transpose(1, 0, 2))

    b1c_h = b1p.reshape(CC, 128).T
    # chunk c: s = c//NT, tile t = c%NT, partition k <-> group 128*t + k
    w2c_h = np.empty((128, CC), dtype=np.float32)
    for c in range(CC):
        w2c_h[:, c] = W2[128 * (c % NT):128 * (c % NT) + 128, c // NT]
    b2c_h = b2.reshape(NT, 128).T
    b3p = b3 + 0.5 * W3.sum(axis=1)  # centering correction for fp8 x2
    b3c_h = b3p.reshape(ND, 128).T
    cst_h = np.ascontiguousarray(
        np.concatenate([b1c_h, w2c_h, b2c_h, b3c_h], axis=1),
        dtype=np.float32)  # [128, 2*CC + NT + ND]
    w3t = (W3.T * W3SCALE).astype(FP8)  # [H2, D_OUT], scaled out of subnormals
    w3q_h = np.ascontiguousarray(
        w3t.reshape(NT, 128, D_OUT).transpose(1, 0, 2))

    in_maps = []
    for i in range(N_CORES):
        in_maps.append({
            "xq": np.ascontiguousarray(
                xq_h[:, :, i * B_SHARD:(i + 1) * B_SHARD]),
            "w1q": w1q_h,
            "cst": cst_h,
            "w3q": w3q_h,
        })
    return in_maps


def kernel(x, W1, b1, W2, b2, W3, b3):
    import os
    from concourse.bass_utils import run_bass_kernel_spmd

    nc = get_nc()
    in_maps = make_in_maps(x, W1, b1, W2, b2, W3, b3)
    # force tracing off for this call: the agent image lacks the axon NTFF
    # hook module, so a stray BASS_TRACE=1 would crash the run
    prev = os.environ.get("BASS_NEVER_TRACE")
    os.environ["BASS_NEVER_TRACE"] = "1"
    try:
        res = run_bass_kernel_spmd(nc, in_maps, core_ids=list(range(N_CORES)))
    finally:
        if prev is None:
            os.environ.pop("BASS_NEVER_TRACE", None)
        else:
            os.environ["BASS_NEVER_TRACE"] = prev
    outT = np.concatenate([res.results[i]["out"].astype(np.float32)
                           for i in range(N_CORES)], axis=1)  # [D_OUT, B]
    return np.ascontiguousarray(outT.T)


# revision 43
# speedup vs baseline: 1.0119x; 1.0119x over previous
"""Trainium2 Bass kernel for AdaptiveNet MLP (fc1+sigmoid, grouped fc2+sigmoid, fc3).

Sharding: pure data-parallel over batch across 8 NeuronCores (no collectives).
Each core computes its 2048-row shard through all three layers.

fc1 (95% of FLOPs) runs in fp8-e4m3 with DoubleRow perf mode (two fp8 weights
per PE cell -> K=256 per matmul, halving the matmul count); the sigmoid damps
the quantization error so the final rel-err stays ~3e-3 (gate is 2e-2).

Layout trick: H1 is permuted s-major on the host (h1' = s*512 + g, where the
original h1 = g*8 + s).  fc1 then produces hT' tiles [128 h1' partitions x 512
rows]; the grouped fc2 contraction over s becomes 8 fused multiply-accumulate
ops on the vector engine with per-partition scalars (W2 columns), and fc3 is a
plain bf16 matmul over the 512 groups.  Biases are per-partition [128,1]
columns fused into ScalarE sigmoids / a VectorE add.
"""

import sys

for _p in ("/opt/trn_rl_repo",):
    if _p not in sys.path:
        sys.path.append(_p)

import numpy as np
import ml_dtypes

BF16 = ml_dtypes.bfloat16
FP8 = ml_dtypes.float8_e4m3  # == mybir.dt.float8e4

D_IN, H1, H2, D_OUT = 1024, 4096, 512, 256
GS = H1 // H2  # 8
B = 16384
N_CORES = 8
B_SHARD = B // N_CORES  # 2048
NBLK = 512  # rows per block (one PSUM bank of fp32)
NB = B_SHARD // NBLK  # 4
KC = D_IN // 128  # 8 contraction subtiles for fc1
KP = KC // 2  # 4 DoubleRow pairs
CC = H1 // 128  # 32 h1' chunks
NT = H2 // 128  # 4 x2T tiles
ND = D_OUT // 128  # 2 output chunks

_compiled = {}


def _build_nc():
    from concourse import bacc, tile, mybir

    f32 = mybir.dt.float32
    bf16 = mybir.dt.bfloat16
    fp8 = mybir.dt.float8e4
    AF = mybir.ActivationFunctionType
    ALU = mybir.AluOpType
    DR = mybir.MatmulPerfMode.DoubleRow

    nc = bacc.Bacc("TRN2", target_bir_lowering=False, debug=False,
                   num_devices=N_CORES)

    xq = nc.dram_tensor("xq", [128, KC, B_SHARD], fp8, kind="ExternalInput")
    w1q = nc.dram_tensor("w1q", [128, KC, H1], fp8, kind="ExternalInput")
    # all [128, *] f32 constants packed on the free axis:
    # b1 (CC) | w2 (CC) | b2 (NT) | b3 (ND)
    cst = nc.dram_tensor("cst", [128, 2 * CC + NT + ND], f32,
                         kind="ExternalInput")
    w3q = nc.dram_tensor("w3q", [128, NT, D_OUT], bf16, kind="ExternalInput")
    out = nc.dram_tensor("out", [D_OUT, B_SHARD], bf16, kind="ExternalOutput")

    with tile.TileContext(nc) as tc:
        with (
            tc.tile_pool(name="wpool", bufs=1) as wpool,
            tc.tile_pool(name="xpool", bufs=1) as xpool,
            tc.tile_pool(name="hpool", bufs=8) as hpool,
            tc.tile_pool(name="accpool", bufs=1) as accpool,
            tc.tile_pool(name="x2pool", bufs=1) as x2pool,
            tc.tile_pool(name="opool", bufs=4) as opool,
            tc.tile_pool(name="psum_h", bufs=8, space="PSUM") as psum_h_pool,
        ):
            psum_o_pool = psum_h_pool
            w1_sb = wpool.tile([128, KC, H1], fp8, tag="w1")
            x_sb = [None] * NB
            for n in range(NB):
                x_sb[n] = xpool.tile([128, KC, NBLK], fp8,
                                     tag=f"x_{n}", name=f"xsb_{n}")
            # Three issuing queues (sync/scalar HWDGE, gpsimd SWDGE), FIFO
            # within each; ~2-3us per dma_start regardless of size, so use
            # few DMAs, landing in exactly the order the loop consumes:
            # W1 column-blocks (all subtile pairs of CBLK c-chunks in one
            # strided DMA) on sync; whole x tiles on scalar/gpsimd.
            CBLK = 4
            WBLK = 8

            cst_sb = wpool.tile([128, 2 * CC + NT + ND], f32, tag="cst")
            w3_sb = wpool.tile([128, NT, D_OUT], bf16, tag="w3q")

            def b1col(c):
                return cst_sb[:, c:c + 1]

            def w2col(c):
                return cst_sb[:, CC + c:CC + c + 1]

            def b2col(t):
                return cst_sb[:, 2 * CC + t:2 * CC + t + 1]

            def b3col(d):
                return cst_sb[:, 2 * CC + NT + d:2 * CC + NT + d + 1]

            # one DMA per W1 column-block covering all subtile pairs; the
            # first block split in half so the very first matmuls unblock
            # sooner
            def wblock(cb0, cb1, eng):
                c0, c1 = cb0 * 128, cb1 * 128
                eng.dma_start(w1_sb[:, :, c0:c1], w1q.ap()[:, :, c0:c1])

            def xdma(n, eng):
                eng.dma_start(x_sb[n][:],
                              xq.ap()[:, :, n * NBLK:(n + 1) * NBLK])

            # sync lane: W1 first half-block x2, then x_1/x_3 slotted in
            # before the later blocks (which have plenty of slack)
            wblock(0, CBLK // 2, nc.sync)
            wblock(CBLK // 2, CBLK, nc.sync)
            xdma(1, nc.sync)
            wblock(CBLK, 2 * CBLK, nc.sync)
            xdma(3, nc.sync)
            for cb in range(2 * CBLK, CC, WBLK):
                wblock(cb, cb + WBLK, nc.sync)
            # scalar/gpsimd lanes: x_0 split across both lanes so the
            # first chunk's matmuls unblock ~1.5us sooner, then x_2/consts
            nc.scalar.dma_start(x_sb[0][:, 0:KC // 2, :],
                                xq.ap()[:, 0:KC // 2, 0:NBLK])
            nc.gpsimd.dma_start(x_sb[0][:, KC // 2:KC, :],
                                xq.ap()[:, KC // 2:KC, 0:NBLK])
            nc.scalar.dma_start(cst_sb[:], cst.ap()[:])
            xdma(2, nc.gpsimd)
            nc.gpsimd.dma_start(w3_sb[:], w3q.ap()[:])

            # fc2 accumulators, one per (row-block, x2 tile)
            acc = [[None] * NT for _ in range(NB)]

            x2_sb = [[None] * NT for _ in range(NB)]

            def fc2_step(c, n, ht):
                t_i = c % NT
                if c < NT:
                    acc[n][t_i] = accpool.tile([128, NBLK], bf16,
                                               tag=f"acc_{n}_{t_i}",
                                               name=f"acc_{n}_{t_i}")
                    nc.vector.tensor_scalar_mul(acc[n][t_i][:], ht[:],
                                                w2col(c))
                else:
                    nc.vector.scalar_tensor_tensor(
                        acc[n][t_i][:], ht[:], w2col(c),
                        acc[n][t_i][:], op0=ALU.mult, op1=ALU.add)
                if c >= CC - NT:
                    # chain for tile t_i is complete -> fc2 sigmoid now so
                    # fc3's t-outer matmuls can start before the last chain
                    t = x2pool.tile([128, NBLK], bf16, tag=f"x2_{n}_{t_i}",
                                    name=f"x2sb_{n}_{t_i}")
                    nc.scalar.activation(t[:], acc[n][t_i][:], AF.Sigmoid,
                                         bias=b2col(t_i))
                    x2_sb[n][t_i] = t

            def fc1_block(cs, n):
                for c in cs:
                    ph = psum_h_pool.tile([128, NBLK], f32, tag="psum_h",
                                          name=f"ph_{n}_{c}")
                    for j in range(KP):
                        nc.tensor.matmul(
                            ph[:],
                            lhsT=w1_sb[:, 2 * j:2 * j + 2,
                                       128 * c:128 * (c + 1)],
                            rhs=x_sb[n][:, 2 * j:2 * j + 2, :],
                            start=(j == 0),
                            stop=(j == KP - 1),
                            perf_mode=DR,
                        )
                    ht = hpool.tile([128, NBLK], bf16, tag="ht",
                                    name=f"ht_{n}_{c}")
                    nc.scalar.activation(ht[:], ph[:], AF.Sigmoid,
                                         bias=b1col(c))
                    fc2_step(c, n, ht)

            def fc3_block(n):
                for d in range(ND):
                    po = psum_o_pool.tile([128, NBLK], f32, tag="psum_h",
                                          name=f"po_{n}_{d}")
                    for t_i in range(NT):
                        nc.tensor.matmul(
                            po[:],
                            lhsT=w3_sb[:, t_i, 128 * d:128 * (d + 1)],
                            rhs=x2_sb[n][t_i][:],
                            start=(t_i == 0),
                            stop=(t_i == NT - 1),
                        )
                    ot = opool.tile([128, NBLK], bf16, tag="ot",
                                    name=f"ot_{n}_{d}")
                    nc.vector.tensor_scalar_add(ot[:], po[:], b3col(d))
                    nc.sync.dma_start(
                        out.ap()[128 * d:128 * (d + 1),
                                 n * NBLK:(n + 1) * NBLK], ot[:])

            # --- fc1 + fc2: column-blocks of CBLK c-chunks, n-outer inside
            # so each x tile's DMA arrival unlocks a block of work; each
            # psum tile's 4 matmuls are consecutive (liveness ~1 bank).
            # In the last block, each row-block's fc3 is interleaved one
            # n-phase behind its fc1 so the x2 sigmoid chains are covered
            # by other matmul work. ---
            NORD = (0, 2, 1, 3)  # x DMA arrival order (2 lanes)
            # chain updates are commutative, so interleave the final c's
            # over the last two blocks: x2 sigmoids spread across 8 chunks
            # instead of bunching behind the last 4 (ACT would throttle PE
            # via PSUM slot release)
            c_seq = list(range(CC - 2 * CBLK)) + [24, 28, 25, 29, 26, 30,
                                                 27, 31]
            for b0 in range(0, CC, CBLK):
                for n in NORD:
                    fc1_block(c_seq[b0:b0 + CBLK], n)
            for n in NORD:
                fc3_block(n)

    nc.compile()
    return nc


def get_nc():
    if "nc" not in _compiled:
        _compiled["nc"] = _build_nc()
    return _compiled["nc"]


def make_in_maps(x, W1, b1, W2, b2, W3, b3):
    x = np.asarray(x, dtype=np.float32)
    W1 = np.asarray(W1, dtype=np.float32)
    b1 = np.asarray(b1, dtype=np.float32)
    W2 = np.asarray(W2, dtype=np.float32)
    b2 = np.asarray(b2, dtype=np.float32)
    W3 = np.asarray(W3, dtype=np.float32)
    b3 = np.asarray(b3, dtype=np.float32)

    # s-major permutation of H1: new index p = s*H2 + g  (old h1 = g*GS + s)
    p = np.arange(H1)
    perm = (p % H2) * GS + (p // H2)
    W1p = W1[perm, :]
    b1p = b1[perm]

    # fp8 fc1 operands in DoubleRow layout [128, KC, *]:
    # element (p, j, m) holds contraction index k = 128*j + p
    w1t = W1p.T.astype(FP8)  # [D_IN, H1]
    w1q_h = np.ascontiguousarray(
        w1t.reshape(KC, 128, H1).transpose(1, 0, 2))
    xt = x.T.astype(FP8)  # [D_IN, B]
    xq_h = np.ascontiguousarray(
        xt.reshape(KC, 128, B).transpose(1, 0, 2))

    b1c_h = b1p.reshape(CC, 128).T
    # chunk c: s = c//NT, tile t = c%NT, partition k <-> group 128*t + k
    w2c_h = np.empty((128, CC), dtype=np.float32)
    for c in range(CC):
        w2c_h[:, c] = W2[128 * (c % NT):128 * (c % NT) + 128, c // NT]
    b2c_h = b2.reshape(NT, 128).T
    b3c_h = b3.reshape(ND, 128).T
    cst_h = np.ascontiguousarray(
        np.concatenate([b1c_h, w2c_h, b2c_h, b3c_h], axis=1),
        dtype=np.float32)  # [128, 2*CC + NT + ND]
    w3t = W3.T.astype(BF16)  # [H2, D_OUT]
    w3q_h = np.ascontiguousarray(
        w3t.reshape(NT, 128, D_OUT).transpose(1, 0, 2))

    in_maps = []
    for i in range(N_CORES):
        in_maps.append({
            "xq": np.ascontiguousarray(
                xq_h[:, :, i * B_SHARD:(i + 1) * B_SHARD]),
            "w1q": w1q_h,
            "cst": cst_h,
            "w3q": w3q_h,
        })
    return in_maps


def kernel(x, W1, b1, W2, b2, W3, b3):
    import os
    from concourse.bass_utils import run_bass_kernel_spmd

    nc = get_nc()
    in_maps = make_in_maps(x, W1, b1, W2, b2, W3, b3)
    # force tracing off for this call: the agent image lacks the axon NTFF
    # hook module, so a stray BASS_TRACE=1 would crash the run
    prev = os.environ.get("BASS_NEVER_TRACE")
    os.environ["BASS_NEVER_TRACE"] = "1"
    try:
        res = run_bass_kernel_spmd(nc, in_maps, core_ids=list(range(N_CORES)))
    finally:
        if prev is None:
            os.environ.pop("BASS_NEVER_TRACE", None)
        else:
            os.environ["BASS_NEVER_TRACE"] = prev
    outT = np.concatenate([res.results[i]["out"].astype(np.float32)
                           for i in range(N_CORES)], axis=1)  # [D_OUT, B]
    return np.ascontiguousarray(outT.T)


# revision 44
# speedup vs baseline: 1.0137x; 1.0018x over previous
"""Trainium2 Bass kernel for AdaptiveNet MLP (fc1+sigmoid, grouped fc2+sigmoid, fc3).

Sharding: pure data-parallel over batch across 8 NeuronCores (no collectives).
Each core computes its 2048-row shard through all three layers.

fc1 (95% of FLOPs) runs in fp8-e4m3 with DoubleRow perf mode (two fp8 weights
per PE cell -> K=256 per matmul, halving the matmul count); the sigmoid damps
the quantization error so the final rel-err stays ~3e-3 (gate is 2e-2).

Layout trick: H1 is permuted s-major on the host (h1' = s*512 + g, where the
original h1 = g*8 + s).  fc1 then produces hT' tiles [128 h1' partitions x 512
rows]; the grouped fc2 contraction over s becomes 8 fused multiply-accumulate
ops on the vector engine with per-partition scalars (W2 columns), and fc3 is a
plain bf16 matmul over the 512 groups.  Biases are per-partition [128,1]
columns fused into ScalarE sigmoids / a VectorE add.
"""

import sys

for _p in ("/opt/trn_rl_repo",):
    if _p not in sys.path:
        sys.path.append(_p)

import numpy as np
import ml_dtypes

BF16 = ml_dtypes.bfloat16
FP8 = ml_dtypes.float8_e4m3  # == mybir.dt.float8e4

D_IN, H1, H2, D_OUT = 1024, 4096, 512, 256
GS = H1 // H2  # 8
B = 16384
N_CORES = 8
B_SHARD = B // N_CORES  # 2048
NBLK = 512  # rows per block (one PSUM bank of fp32)
NB = B_SHARD // NBLK  # 4
KC = D_IN // 128  # 8 contraction subtiles for fc1
KP = KC // 2  # 4 DoubleRow pairs
CC = H1 // 128  # 32 h1' chunks
NT = H2 // 128  # 4 x2T tiles
ND = D_OUT // 128  # 2 output chunks

_compiled = {}


def _build_nc():
    from concourse import bacc, tile, mybir

    f32 = mybir.dt.float32
    bf16 = mybir.dt.bfloat16
    fp8 = mybir.dt.float8e4
    AF = mybir.ActivationFunctionType
    ALU = mybir.AluOpType
    DR = mybir.MatmulPerfMode.DoubleRow

    nc = bacc.Bacc("TRN2", target_bir_lowering=False, debug=False,
                   num_devices=N_CORES)

    xq = nc.dram_tensor("xq", [128, KC, B_SHARD], fp8, kind="ExternalInput")
    w1q = nc.dram_tensor("w1q", [128, KC, H1], fp8, kind="ExternalInput")
    # all [128, *] f32 constants packed on the free axis:
    # b1 (CC) | w2 (CC) | b2 (NT) | b3 (ND)
    cst = nc.dram_tensor("cst", [128, 2 * CC + NT + ND], f32,
                         kind="ExternalInput")
    w3q = nc.dram_tensor("w3q", [128, NT, D_OUT], bf16, kind="ExternalInput")
    out = nc.dram_tensor("out", [D_OUT, B_SHARD], bf16, kind="ExternalOutput")

    with tile.TileContext(nc) as tc:
        with (
            tc.tile_pool(name="wpool", bufs=1) as wpool,
            tc.tile_pool(name="xpool", bufs=1) as xpool,
            tc.tile_pool(name="hpool", bufs=8) as hpool,
            tc.tile_pool(name="accpool", bufs=1) as accpool,
            tc.tile_pool(name="x2pool", bufs=1) as x2pool,
            tc.tile_pool(name="opool", bufs=4) as opool,
            tc.tile_pool(name="psum_h", bufs=8, space="PSUM") as psum_h_pool,
        ):
            psum_o_pool = psum_h_pool
            w1_sb = wpool.tile([128, KC, H1], fp8, tag="w1")
            x_sb = [None] * NB
            for n in range(NB):
                x_sb[n] = xpool.tile([128, KC, NBLK], fp8,
                                     tag=f"x_{n}", name=f"xsb_{n}")
            # Three issuing queues (sync/scalar HWDGE, gpsimd SWDGE), FIFO
            # within each; ~2-3us per dma_start regardless of size, so use
            # few DMAs, landing in exactly the order the loop consumes:
            # W1 column-blocks (all subtile pairs of CBLK c-chunks in one
            # strided DMA) on sync; whole x tiles on scalar/gpsimd.
            CBLK = 4
            WBLK = 8

            cst_sb = wpool.tile([128, 2 * CC + NT + ND], f32, tag="cst")
            w3_sb = wpool.tile([128, NT, D_OUT], bf16, tag="w3q")

            def b1col(c):
                return cst_sb[:, c:c + 1]

            def w2col(c):
                return cst_sb[:, CC + c:CC + c + 1]

            def b2col(t):
                return cst_sb[:, 2 * CC + t:2 * CC + t + 1]

            def b3col(d):
                return cst_sb[:, 2 * CC + NT + d:2 * CC + NT + d + 1]

            # one DMA per W1 column-block covering all subtile pairs; the
            # first block split in half so the very first matmuls unblock
            # sooner
            def wblock(cb0, cb1, eng):
                c0, c1 = cb0 * 128, cb1 * 128
                eng.dma_start(w1_sb[:, :, c0:c1], w1q.ap()[:, :, c0:c1])

            def xdma(n, eng):
                eng.dma_start(x_sb[n][:],
                              xq.ap()[:, :, n * NBLK:(n + 1) * NBLK])

            # sync lane: W1 first half-block x2, then x_1/x_3 slotted in
            # before the later blocks (which have plenty of slack)
            wblock(0, 1, nc.sync)
            wblock(1, CBLK, nc.sync)
            xdma(1, nc.sync)
            wblock(CBLK, 2 * CBLK, nc.sync)
            xdma(3, nc.sync)
            for cb in range(2 * CBLK, CC, WBLK):
                wblock(cb, cb + WBLK, nc.sync)
            # scalar/gpsimd lanes: x_0 split across both lanes so the
            # first chunk's matmuls unblock ~1.5us sooner, then x_2/consts
            nc.scalar.dma_start(x_sb[0][:, 0:KC // 2, :],
                                xq.ap()[:, 0:KC // 2, 0:NBLK])
            nc.gpsimd.dma_start(x_sb[0][:, KC // 2:KC, :],
                                xq.ap()[:, KC // 2:KC, 0:NBLK])
            nc.scalar.dma_start(cst_sb[:], cst.ap()[:])
            xdma(2, nc.gpsimd)
            nc.gpsimd.dma_start(w3_sb[:], w3q.ap()[:])

            # fc2 accumulators, one per (row-block, x2 tile)
            acc = [[None] * NT for _ in range(NB)]

            x2_sb = [[None] * NT for _ in range(NB)]

            def fc2_step(c, n, ht):
                t_i = c % NT
                if c < NT:
                    acc[n][t_i] = accpool.tile([128, NBLK], bf16,
                                               tag=f"acc_{n}_{t_i}",
                                               name=f"acc_{n}_{t_i}")
                    nc.vector.tensor_scalar_mul(acc[n][t_i][:], ht[:],
                                                w2col(c))
                else:
                    nc.vector.scalar_tensor_tensor(
                        acc[n][t_i][:], ht[:], w2col(c),
                        acc[n][t_i][:], op0=ALU.mult, op1=ALU.add)
                if c >= CC - NT:
                    # chain for tile t_i is complete -> fc2 sigmoid now so
                    # fc3's t-outer matmuls can start before the last chain
                    t = x2pool.tile([128, NBLK], bf16, tag=f"x2_{n}_{t_i}",
                                    name=f"x2sb_{n}_{t_i}")
                    nc.scalar.activation(t[:], acc[n][t_i][:], AF.Sigmoid,
                                         bias=b2col(t_i))
                    x2_sb[n][t_i] = t

            def fc1_block(cs, n):
                for c in cs:
                    ph = psum_h_pool.tile([128, NBLK], f32, tag="psum_h",
                                          name=f"ph_{n}_{c}")
                    for j in range(KP):
                        nc.tensor.matmul(
                            ph[:],
                            lhsT=w1_sb[:, 2 * j:2 * j + 2,
                                       128 * c:128 * (c + 1)],
                            rhs=x_sb[n][:, 2 * j:2 * j + 2, :],
                            start=(j == 0),
                            stop=(j == KP - 1),
                            perf_mode=DR,
                        )
                    ht = hpool.tile([128, NBLK], bf16, tag="ht",
                                    name=f"ht_{n}_{c}")
                    nc.scalar.activation(ht[:], ph[:], AF.Sigmoid,
                                         bias=b1col(c))
                    fc2_step(c, n, ht)

            def fc3_block(n, last=False):
                for d in range(ND):
                    po = psum_o_pool.tile([128, NBLK], f32, tag="psum_h",
                                          name=f"po_{n}_{d}")
                    for t_i in range(NT):
                        nc.tensor.matmul(
                            po[:],
                            lhsT=w3_sb[:, t_i, 128 * d:128 * (d + 1)],
                            rhs=x2_sb[n][t_i][:],
                            start=(t_i == 0),
                            stop=(t_i == NT - 1),
                        )
                    ot = opool.tile([128, NBLK], bf16, tag="ot",
                                    name=f"ot_{n}_{d}")
                    if last and d == ND - 1:
                        # final output: two half-DMAs on separate lanes so
                        # the end-of-kernel exposure is one 64KB transfer
                        H = NBLK // 2
                        nc.vector.tensor_scalar_add(ot[:, 0:H], po[:, 0:H],
                                                    b3col(d))
                        nc.sync.dma_start(
                            out.ap()[128 * d:128 * (d + 1),
                                     n * NBLK:n * NBLK + H], ot[:, 0:H])
                        nc.vector.tensor_scalar_add(ot[:, H:NBLK],
                                                    po[:, H:NBLK], b3col(d))
                        nc.scalar.dma_start(
                            out.ap()[128 * d:128 * (d + 1),
                                     n * NBLK + H:(n + 1) * NBLK],
                            ot[:, H:NBLK])
                    else:
                        nc.vector.tensor_scalar_add(ot[:], po[:], b3col(d))
                        nc.sync.dma_start(
                            out.ap()[128 * d:128 * (d + 1),
                                     n * NBLK:(n + 1) * NBLK], ot[:])

            # --- fc1 + fc2: column-blocks of CBLK c-chunks, n-outer inside
            # so each x tile's DMA arrival unlocks a block of work; each
            # psum tile's 4 matmuls are consecutive (liveness ~1 bank).
            # In the last block, each row-block's fc3 is interleaved one
            # n-phase behind its fc1 so the x2 sigmoid chains are covered
            # by other matmul work. ---
            NORD = (0, 2, 1, 3)  # x DMA arrival order (2 lanes)
            # chain updates are commutative, so interleave the final c's
            # over the last two blocks: x2 sigmoids spread across 8 chunks
            # instead of bunching behind the last 4 (ACT would throttle PE
            # via PSUM slot release)
            c_seq = list(range(CC - 2 * CBLK)) + [24, 28, 25, 29, 26, 30,
                                                 27, 31]
            for b0 in range(0, CC, CBLK):
                for n in NORD:
                    fc1_block(c_seq[b0:b0 + CBLK], n)
            for n in NORD:
                fc3_block(n, last=(n == NORD[-1]))

    nc.compile()
    return nc


def get_nc():
    if "nc" not in _compiled:
        _compiled["nc"] = _build_nc()
    return _compiled["nc"]


def make_in_maps(x, W1, b1, W2, b2, W3, b3):
    x = np.asarray(x, dtype=np.float32)
    W1 = np.asarray(W1, dtype=np.float32)
    b1 = np.asarray(b1, dtype=np.float32)
    W2 = np.asarray(W2, dtype=np.float32)
    b2 = np.asarray(b2, dtype=np.float32)
    W3 = np.asarray(W3, dtype=np.float32)
    b3 = np.asarray(b3, dtype=np.float32)

    # s-major permutation of H1: new index p = s*H2 + g  (old h1 = g*GS + s)
    p = np.arange(H1)
    perm = (p % H2) * GS + (p // H2)
    W1p = W1[perm, :]
    b1p = b1[perm]

    # fp8 fc1 operands in DoubleRow layout [128, KC, *]:
    # element (p, j, m) holds contraction index k = 128*j + p
    w1t = W1p.T.astype(FP8)  # [D_IN, H1]
    w1q_h = np.ascontiguousarray(
        w1t.reshape(KC, 128, H1).transpose(1, 0, 2))
    xt = x.T.astype(FP8)  # [D_IN, B]
    xq_h = np.ascontiguousarray(
        xt.reshape(KC, 128, B).transpose(1, 0, 2))

    b1c_h = b1p.reshape(CC, 128).T
    # chunk c: s = c//NT, tile t = c%NT, partition k <-> group 128*t + k
    w2c_h = np.empty((128, CC), dtype=np.float32)
    for c in range(CC):
        w2c_h[:, c] = W2[128 * (c % NT):128 * (c % NT) + 128, c // NT]
    b2c_h = b2.reshape(NT, 128).T
    b3c_h = b3.reshape(ND, 128).T
    cst_h = np.ascontiguousarray(
        np.concatenate([b1c_h, w2c_h, b2c_h, b3c_h], axis=1),
        dtype=np.float32)  # [128, 2*CC + NT + ND]
    w3t = W3.T.astype(BF16)  # [H2, D_OUT]
    w3q_h = np.ascontiguousarray(
        w3t.reshape(NT, 128, D_OUT).transpose(1, 0, 2))

    in_maps = []
    for i in range(N_CORES):
        in_maps.append({
            "xq": np.ascontiguousarray(
                xq_h[:, :, i * B_SHARD:(i + 1) * B_SHARD]),
            "w1q": w1q_h,
            "cst": cst_h,
            "w3q": w3q_h,
        })
    return in_maps


def kernel(x, W1, b1, W2, b2, W3, b3):
    import os
    from concourse.bass_utils import run_bass_kernel_spmd

    nc = get_nc()
    in_maps = make_in_maps(x, W1, b1, W2, b2, W3, b3)
    # force tracing off for this call: the agent image lacks the axon NTFF
    # hook module, so a stray BASS_TRACE=1 would crash the run
    prev = os.environ.get("BASS_NEVER_TRACE")
    os.environ["BASS_NEVER_TRACE"] = "1"
    try:
        res = run_bass_kernel_spmd(nc, in_maps, core_ids=list(range(N_CORES)))
    finally:
        if prev is None:
            os.environ.pop("BASS_NEVER_TRACE", None)
        else:
            os.environ["BASS_NEVER_TRACE"] = prev
    outT = np.concatenate([res.results[i]["out"].astype(np.float32)
                           for i in range(N_CORES)], axis=1)  # [D_OUT, B]
    return np.ascontiguousarray(outT.T)


# revision 45
# speedup vs baseline: 1.0158x; 1.0021x over previous
"""Trainium2 Bass kernel for AdaptiveNet MLP (fc1+sigmoid, grouped fc2+sigmoid, fc3).

Sharding: pure data-parallel over batch across 8 NeuronCores (no collectives).
Each core computes its 2048-row shard through all three layers.

fc1 (95% of FLOPs) runs in fp8-e4m3 with DoubleRow perf mode (two fp8 weights
per PE cell -> K=256 per matmul, halving the matmul count); the sigmoid damps
the quantization error so the final rel-err stays ~3e-3 (gate is 2e-2).

Layout trick: H1 is permuted s-major on the host (h1' = s*512 + g, where the
original h1 = g*8 + s).  fc1 then produces hT' tiles [128 h1' partitions x 512
rows]; the grouped fc2 contraction over s becomes 8 fused multiply-accumulate
ops on the vector engine with per-partition scalars (W2 columns), and fc3 is a
plain bf16 matmul over the 512 groups.  Biases are per-partition [128,1]
columns fused into ScalarE sigmoids / a VectorE add.
"""

import sys

for _p in ("/opt/trn_rl_repo",):
    if _p not in sys.path:
        sys.path.append(_p)

import numpy as np
import ml_dtypes

BF16 = ml_dtypes.bfloat16
FP8 = ml_dtypes.float8_e4m3  # == mybir.dt.float8e4

D_IN, H1, H2, D_OUT = 1024, 4096, 512, 256
GS = H1 // H2  # 8
B = 16384
N_CORES = 8
B_SHARD = B // N_CORES  # 2048
NBLK = 512  # rows per block (one PSUM bank of fp32)
NB = B_SHARD // NBLK  # 4
KC = D_IN // 128  # 8 contraction subtiles for fc1
KP = KC // 2  # 4 DoubleRow pairs
CC = H1 // 128  # 32 h1' chunks
NT = H2 // 128  # 4 x2T tiles
ND = D_OUT // 128  # 2 output chunks
W3SCALE = 16.0  # W3 pre-scale (fp8 subnormal avoidance); undone in bias add

_compiled = {}


def _build_nc():
    from concourse import bacc, tile, mybir

    f32 = mybir.dt.float32
    bf16 = mybir.dt.bfloat16
    fp8 = mybir.dt.float8e4
    AF = mybir.ActivationFunctionType
    ALU = mybir.AluOpType
    DR = mybir.MatmulPerfMode.DoubleRow

    nc = bacc.Bacc("TRN2", target_bir_lowering=False, debug=False,
                   num_devices=N_CORES)

    xq = nc.dram_tensor("xq", [128, KC, B_SHARD], fp8, kind="ExternalInput")
    w1q = nc.dram_tensor("w1q", [128, KC, H1], fp8, kind="ExternalInput")
    # all [128, *] f32 constants packed on the free axis:
    # b1 (CC) | w2 (CC) | b2 (NT) | b3 (ND)
    cst = nc.dram_tensor("cst", [128, 2 * CC + NT + ND], f32,
                         kind="ExternalInput")
    w3a = nc.dram_tensor("w3a", [128, 2, D_OUT], fp8, kind="ExternalInput")
    w3b = nc.dram_tensor("w3b", [128, 2, D_OUT], bf16, kind="ExternalInput")
    out = nc.dram_tensor("out", [D_OUT, B_SHARD], bf16, kind="ExternalOutput")

    with tile.TileContext(nc) as tc:
        with (
            tc.tile_pool(name="wpool", bufs=1) as wpool,
            tc.tile_pool(name="xpool", bufs=1) as xpool,
            tc.tile_pool(name="hpool", bufs=8) as hpool,
            tc.tile_pool(name="accpool", bufs=1) as accpool,
            tc.tile_pool(name="x2pool", bufs=1) as x2pool,
            tc.tile_pool(name="opool", bufs=4) as opool,
            tc.tile_pool(name="psum_h", bufs=8, space="PSUM") as psum_h_pool,
        ):
            psum_o_pool = psum_h_pool
            w1_sb = wpool.tile([128, KC, H1], fp8, tag="w1")
            x_sb = [None] * NB
            for n in range(NB):
                x_sb[n] = xpool.tile([128, KC, NBLK], fp8,
                                     tag=f"x_{n}", name=f"xsb_{n}")
            # Three issuing queues (sync/scalar HWDGE, gpsimd SWDGE), FIFO
            # within each; ~2-3us per dma_start regardless of size, so use
            # few DMAs, landing in exactly the order the loop consumes:
            # W1 column-blocks (all subtile pairs of CBLK c-chunks in one
            # strided DMA) on sync; whole x tiles on scalar/gpsimd.
            CBLK = 4
            WBLK = 8

            cst_sb = wpool.tile([128, 2 * CC + NT + ND], f32, tag="cst")
            w3a_sb = wpool.tile([128, 2, D_OUT], fp8, tag="w3a")
            w3b_sb = wpool.tile([128, 2, D_OUT], bf16, tag="w3b")

            def b1col(c):
                return cst_sb[:, c:c + 1]

            def w2col(c):
                return cst_sb[:, CC + c:CC + c + 1]

            def b2col(t):
                return cst_sb[:, 2 * CC + t:2 * CC + t + 1]

            def b3col(d):
                return cst_sb[:, 2 * CC + NT + d:2 * CC + NT + d + 1]

            # one DMA per W1 column-block covering all subtile pairs; the
            # first block split in half so the very first matmuls unblock
            # sooner
            def wblock(cb0, cb1, eng):
                c0, c1 = cb0 * 128, cb1 * 128
                eng.dma_start(w1_sb[:, :, c0:c1], w1q.ap()[:, :, c0:c1])

            def xdma(n, eng):
                eng.dma_start(x_sb[n][:],
                              xq.ap()[:, :, n * NBLK:(n + 1) * NBLK])

            # sync lane: W1 first half-block x2, then x_1/x_3 slotted in
            # before the later blocks (which have plenty of slack)
            wblock(0, 1, nc.sync)
            wblock(1, CBLK, nc.sync)
            xdma(1, nc.sync)
            wblock(CBLK, 2 * CBLK, nc.sync)
            xdma(3, nc.sync)
            for cb in range(2 * CBLK, CC, WBLK):
                wblock(cb, cb + WBLK, nc.sync)
            # scalar/gpsimd lanes: x_0 split across both lanes so the
            # first chunk's matmuls unblock ~1.5us sooner, then x_2/consts
            nc.scalar.dma_start(x_sb[0][:, 0:KC // 2, :],
                                xq.ap()[:, 0:KC // 2, 0:NBLK])
            nc.gpsimd.dma_start(x_sb[0][:, KC // 2:KC, :],
                                xq.ap()[:, KC // 2:KC, 0:NBLK])
            nc.scalar.dma_start(cst_sb[:], cst.ap()[:])
            xdma(2, nc.gpsimd)
            nc.gpsimd.dma_start(w3a_sb[:], w3a.ap()[:])
            nc.gpsimd.dma_start(w3b_sb[:], w3b.ap()[:])

            # fc2 accumulators, one per (row-block, x2 tile)
            acc = [[None] * NT for _ in range(NB)]

            x2_sb = [[None] * NT for _ in range(NB)]
            x2p_sb = [None] * NB
            for n in range(NB):
                x2p_sb[n] = x2pool.tile([128, 2, NBLK], fp8,
                                        tag=f"x2p_{n}", name=f"x2p_{n}")

            def fc2_step(c, n, ht):
                t_i = c % NT
                if c < NT:
                    acc[n][t_i] = accpool.tile([128, NBLK], bf16,
                                               tag=f"acc_{n}_{t_i}",
                                               name=f"acc_{n}_{t_i}")
                    nc.vector.tensor_scalar_mul(acc[n][t_i][:], ht[:],
                                                w2col(c))
                else:
                    nc.vector.scalar_tensor_tensor(
                        acc[n][t_i][:], ht[:], w2col(c),
                        acc[n][t_i][:], op0=ALU.mult, op1=ALU.add)
                if c >= CC - NT:
                    # chain for tile t_i is complete -> fc2 sigmoid now.
                    # t0/t1 finish early: center by -0.5 into a packed fp8
                    # tile for a single DoubleRow fc3 matmul (the DVE hop
                    # is hidden mid-kernel). t2/t3 finish last: keep the
                    # short bf16 ACT-only chain.
                    if t_i < 2:
                        x2h = hpool.tile([128, NBLK], bf16, tag="x2h",
                                         name=f"x2h_{n}_{t_i}")
                        nc.scalar.activation(x2h[:], acc[n][t_i][:],
                                             AF.Sigmoid, bias=b2col(t_i))
                        nc.vector.tensor_scalar_sub(
                            x2p_sb[n][:, t_i, :], x2h[:], 0.5)
                    else:
                        t = x2pool.tile([128, NBLK], bf16,
                                        tag=f"x2_{n}_{t_i}",
                                        name=f"x2sb_{n}_{t_i}")
                        nc.scalar.activation(t[:], acc[n][t_i][:],
                                             AF.Sigmoid, bias=b2col(t_i))
                        x2_sb[n][t_i] = t

            def fc1_block(cs, n):
                for c in cs:
                    ph = psum_h_pool.tile([128, NBLK], f32, tag="psum_h",
                                          name=f"ph_{n}_{c}")
                    for j in range(KP):
                        nc.tensor.matmul(
                            ph[:],
                            lhsT=w1_sb[:, 2 * j:2 * j + 2,
                                       128 * c:128 * (c + 1)],
                            rhs=x_sb[n][:, 2 * j:2 * j + 2, :],
                            start=(j == 0),
                            stop=(j == KP - 1),
                            perf_mode=DR,
                        )
                    ht = hpool.tile([128, NBLK], bf16, tag="ht",
                                    name=f"ht_{n}_{c}")
                    nc.scalar.activation(ht[:], ph[:], AF.Sigmoid,
                                         bias=b1col(c))
                    fc2_step(c, n, ht)

            def fc3_block(n, last=False):
                for d in range(ND):
                    po = psum_o_pool.tile([128, NBLK], f32, tag="psum_h",
                                          name=f"po_{n}_{d}")
                    nc.tensor.matmul(
                        po[:],
                        lhsT=w3a_sb[:, :, 128 * d:128 * (d + 1)],
                        rhs=x2p_sb[n][:],
                        start=True, stop=False, perf_mode=DR,
                        skip_group_check=True,
                    )
                    for t_i in (2, 3):
                        nc.tensor.matmul(
                            po[:],
                            lhsT=w3b_sb[:, t_i - 2, 128 * d:128 * (d + 1)],
                            rhs=x2_sb[n][t_i][:],
                            start=False,
                            stop=(t_i == 3),
                            skip_group_check=True,
                        )
                    ot = opool.tile([128, NBLK], bf16, tag="ot",
                                    name=f"ot_{n}_{d}")
                    if last and d == ND - 1:
                        # final output: two half-DMAs on separate lanes so
                        # the end-of-kernel exposure is one 64KB transfer
                        H = NBLK // 2
                        nc.vector.tensor_scalar(ot[:, 0:H], po[:, 0:H],
                                                1.0 / W3SCALE, b3col(d),
                                                op0=ALU.mult, op1=ALU.add)
                        nc.sync.dma_start(
                            out.ap()[128 * d:128 * (d + 1),
                                     n * NBLK:n * NBLK + H], ot[:, 0:H])
                        nc.vector.tensor_scalar(ot[:, H:NBLK],
                                                po[:, H:NBLK],
                                                1.0 / W3SCALE, b3col(d),
                                                op0=ALU.mult, op1=ALU.add)
                        nc.scalar.dma_start(
                            out.ap()[128 * d:128 * (d + 1),
                                     n * NBLK + H:(n + 1) * NBLK],
                            ot[:, H:NBLK])
                    else:
                        nc.vector.tensor_scalar(ot[:], po[:],
                                                1.0 / W3SCALE, b3col(d),
                                                op0=ALU.mult, op1=ALU.add)
                        nc.sync.dma_start(
                            out.ap()[128 * d:128 * (d + 1),
                                     n * NBLK:(n + 1) * NBLK], ot[:])

            # --- fc1 + fc2: column-blocks of CBLK c-chunks, n-outer inside
            # so each x tile's DMA arrival unlocks a block of work; each
            # psum tile's 4 matmuls are consecutive (liveness ~1 bank).
            # In the last block, each row-block's fc3 is interleaved one
            # n-phase behind its fc1 so the x2 sigmoid chains are covered
            # by other matmul work. ---
            NORD = (0, 2, 1, 3)  # x DMA arrival order (2 lanes)
            # chain updates are commutative, so interleave the final c's
            # over the last two blocks: x2 sigmoids spread across 8 chunks
            # instead of bunching behind the last 4 (ACT would throttle PE
            # via PSUM slot release)
            c_seq = list(range(CC - 2 * CBLK)) + [24, 28, 25, 29, 26, 30,
                                                 27, 31]
            for b0 in range(0, CC, CBLK):
                for n in NORD:
                    fc1_block(c_seq[b0:b0 + CBLK], n)
            for n in NORD:
                fc3_block(n, last=(n == NORD[-1]))

    nc.compile()
    return nc


def get_nc():
    if "nc" not in _compiled:
        _compiled["nc"] = _build_nc()
    return _compiled["nc"]


def make_in_maps(x, W1, b1, W2, b2, W3, b3):
    x = np.asarray(x, dtype=np.float32)
    W1 = np.asarray(W1, dtype=np.float32)
    b1 = np.asarray(b1, dtype=np.float32)
    W2 = np.asarray(W2, dtype=np.float32)
    b2 = np.asarray(b2, dtype=np.float32)
    W3 = np.asarray(W3, dtype=np.float32)
    b3 = np.asarray(b3, dtype=np.float32)

    # s-major permutation of H1: new index p = s*H2 + g  (old h1 = g*GS + s)
    p = np.arange(H1)
    perm = (p % H2) * GS + (p // H2)
    W1p = W1[perm, :]
    b1p = b1[perm]

    # fp8 fc1 operands in DoubleRow layout [128, KC, *]:
    # element (p, j, m) holds contraction index k = 128*j + p
    w1t = W1p.T.astype(FP8)  # [D_IN, H1]
    w1q_h = np.ascontiguousarray(
        w1t.reshape(KC, 128, H1).transpose(1, 0, 2))
    xt = x.T.astype(FP8)  # [D_IN, B]
    xq_h = np.ascontiguousarray(
        xt.reshape(KC, 128, B).transpose(1, 0, 2))

    b1c_h = b1p.reshape(CC, 128).T
    # chunk c: s = c//NT, tile t = c%NT, partition k <-> group 128*t + k
    w2c_h = np.empty((128, CC), dtype=np.float32)
    for c in range(CC):
        w2c_h[:, c] = W2[128 * (c % NT):128 * (c % NT) + 128, c // NT]
    b2c_h = b2.reshape(NT, 128).T
    # fc3: t0/t1 half in fp8 with x2 centered by -0.5 (correction folded
    # into b3); both halves 16x-scaled, undone in the final fused add
    b3p = b3 + 0.5 * W3[:, 0:256].sum(axis=1)
    b3c_h = b3p.reshape(ND, 128).T
    cst_h = np.ascontiguousarray(
        np.concatenate([b1c_h, w2c_h, b2c_h, b3c_h], axis=1),
        dtype=np.float32)  # [128, 2*CC + NT + ND]
    w3t = W3.T * W3SCALE  # [H2, D_OUT]
    w3a_h = np.ascontiguousarray(
        w3t[0:256].astype(FP8).reshape(2, 128, D_OUT).transpose(1, 0, 2))
    w3b_h = np.ascontiguousarray(
        w3t[256:512].astype(BF16).reshape(2, 128, D_OUT).transpose(1, 0, 2))

    in_maps = []
    for i in range(N_CORES):
        in_maps.append({
            "xq": np.ascontiguousarray(
                xq_h[:, :, i * B_SHARD:(i + 1) * B_SHARD]),
            "w1q": w1q_h,
            "cst": cst_h,
            "w3a": w3a_h,
            "w3b": w3b_h,
        })
    return in_maps


def kernel(x, W1, b1, W2, b2, W3, b3):
    import os
    from concourse.bass_utils import run_bass_kernel_spmd

    nc = get_nc()
    in_maps = make_in_maps(x, W1, b1, W2, b2, W3, b3)
    # force tracing off for this call: the agent image lacks the axon NTFF
    # hook module, so a stray BASS_TRACE=1 would crash the run
    prev = os.environ.get("BASS_NEVER_TRACE")
    os.environ["BASS_NEVER_TRACE"] = "1"
    try:
        res = run_bass_kernel_spmd(nc, in_maps, core_ids=list(range(N_CORES)))
    finally:
        if prev is None:
            os.environ.pop("BASS_NEVER_TRACE", None)
        else:
            os.environ["BASS_NEVER_TRACE"] = prev
    outT = np.concatenate([res.results[i]["out"].astype(np.float32)
                           for i in range(N_CORES)], axis=1)  # [D_OUT, B]
    return np.ascontiguousarray(outT.T)
